# revision 1
# baseline (speedup 1.0000x reference)
"""Dual-path multi-head attention on 8 trn2 NeuronCores.

Sharding: core c = (path p=c//4, batch b=c%4). Each core runs the full
pipeline for one path and one batch element: 3 input projections, 16-head
attention (S=1024, dh=64), output projection. No collectives.

Path 2 cross-wiring (q2 from k; k2,v2 from q) is handled purely by host-side
input routing - every core runs the identical SPMD program.

Device layouts (per core, all pre-packed on host for contiguous DMA runs):
  xq/xk/xv : [p, n, s]   = x.T blocked:  x[s, n*128+p]
  wq/wc    : [p, m, n, e'] = W[m*128+e', n*128+p]  (W.T blocked by out-block m)
  wv       : [p, n, e]   = Wv[e, n*128+p]
  Projections compute Q1T/K1T = [e, s] and V1 = [s, e]; scores are computed
  transposed (probs_T[sk, sq]) so softmax needs no transposes. Softmax is
  max-free (scores ~ N(0,1)); the denominator comes from a ones-column
  appended per head slot in v1e (PV yields [dh+1, sq], row 64 = sum).
Emission order software-pipelines Q/K projections of block m+1 between the
two attention heads of block m so their PSUM->SBUF drains hide under PE work.
"""

import numpy as np
import ml_dtypes

B, S, D, H, DH = 4, 1024, 1024, 16, 64
NB = D // 128  # 8 partition-blocks
HW = 65  # head slot width in v1e (64 data + 1 ones col)

_compiled = None


def _build():
    import concourse.bass as bass
    import concourse.mybir as mybir
    import concourse.tile as tile
    from concourse import bacc

    dt = mybir.dt
    f32, bf16, f32r = dt.float32, dt.bfloat16, dt.float32r

    nc = bacc.Bacc("TRN2", target_bir_lowering=False, debug=False)

    xq_d = nc.dram_tensor("xq", [128, NB, S], bf16, kind="ExternalInput")
    xk_d = nc.dram_tensor("xk", [128, NB, S], bf16, kind="ExternalInput")
    xv_d = nc.dram_tensor("xv", [128, NB, S], bf16, kind="ExternalInput")
    wq_d = nc.dram_tensor("wq", [128, NB, NB, 128], bf16, kind="ExternalInput")
    wk_d = nc.dram_tensor("wk", [128, NB, NB, 128], bf16, kind="ExternalInput")
    wv_d = nc.dram_tensor("wv", [128, NB, D], bf16, kind="ExternalInput")
    wc_d = nc.dram_tensor("wc", [128, NB, NB, 128], bf16, kind="ExternalInput")
    bq_d = nc.dram_tensor("bq", [128, NB], f32, kind="ExternalInput")
    bk_d = nc.dram_tensor("bk", [128, NB], f32, kind="ExternalInput")
    bc_d = nc.dram_tensor("bc", [128, NB], f32, kind="ExternalInput")
    bvB_d = nc.dram_tensor("bvB", [128, D], bf16, kind="ExternalInput")
    out_d = nc.dram_tensor("outT", [D, S], f32, kind="ExternalOutput")
    rdram = nc.dram_tensor("rbounce", [H, S], f32)

    ExpF = mybir.ActivationFunctionType.Exp

    with tile.TileContext(nc) as tc:
        with tc.tile_pool(name="x", bufs=3) as xp, \
             tc.tile_pool(name="wfull", bufs=1) as wfp, \
             tc.tile_pool(name="wblk", bufs=4) as wbp, \
             tc.tile_pool(name="cst", bufs=1) as cp, \
             tc.tile_pool(name="qk", bufs=4) as qkp, \
             tc.tile_pool(name="pers", bufs=1) as prp, \
             tc.tile_pool(name="pt", bufs=2) as ptp, \
             tc.tile_pool(name="stage", bufs=2) as stp, \
             tc.tile_pool(name="rcp", bufs=2) as rcp, \
             tc.tile_pool(name="ost", bufs=2) as ostp, \
             tc.tile_pool(name="mm", bufs=2, space="PSUM") as mmp, \
             tc.tile_pool(name="vp", bufs=2, space="PSUM") as vpp:

            # ---- loads: first V-proj blocks (interleaved for early start),
            # constants after the first block pair, then xq/xk; wc last.
            xv_t = xp.tile([128, NB, S], bf16, tag="x")
            wv_t = wfp.tile([128, NB, D], bf16)
            nc.sync.dma_start(out=xv_t[:, 0, :], in_=xv_d.ap()[:, 0, :])
            nc.sync.dma_start(out=wv_t[:, 0, :], in_=wv_d.ap()[:, 0, :])
            bq_t = cp.tile([128, NB], f32)
            nc.sync.dma_start(out=bq_t[:, :], in_=bq_d.ap())
            bk_t = cp.tile([128, NB], f32)
            nc.sync.dma_start(out=bk_t[:, :], in_=bk_d.ap())
            bc_t = cp.tile([128, NB], f32)
            nc.sync.dma_start(out=bc_t[:, :], in_=bc_d.ap())
            bvB_t = cp.tile([128, D], bf16)
            nc.sync.dma_start(out=bvB_t[:, :], in_=bvB_d.ap())
            ones64 = cp.tile([65, 64], bf16)
            nc.vector.memset(ones64[:, :], 1.0)
            for n in range(1, NB):
                nc.sync.dma_start(out=xv_t[:, n, :], in_=xv_d.ap()[:, n, :])
                nc.sync.dma_start(out=wv_t[:, n, :], in_=wv_d.ap()[:, n, :])
            xq_t = xp.tile([128, NB, S], bf16, tag="x")
            nc.sync.dma_start(out=xq_t[:, :, :], in_=xq_d.ap())
            xk_t = xp.tile([128, NB, S], bf16, tag="x")
            nc.sync.dma_start(out=xk_t[:, :, :], in_=xk_d.ap())

            v1e = prp.tile([128, NB, H * HW], bf16)
            a1 = [prp.tile([128, S], bf16, tag=f"a1_{n}", name=f"a1_{n}")
                  for n in range(NB)]

            # ones columns of v1e (softmax denominator trick)
            ones_ap = v1e[:, :, :].rearrange("p n (h x) -> p n h x", x=HW)[:, :, :, 64]
            nc.vector.memset(ones_ap, 1.0)

            def vproj_block(n2):
                ps = vpp.tile([128, 2, 512], f32, tag="vp", name=f"vps{n2}")
                for n in range(NB):
                    for c in range(2):
                        nc.tensor.matmul(
                            ps[:, c, :],
                            xv_t[:, n, n2 * 128:(n2 + 1) * 128],
                            wv_t[:, n, c * 512:(c + 1) * 512],
                            start=(n == 0), stop=(n == NB - 1),
                        )
                dst = v1e[:, n2, :].rearrange("p (c h x) -> p c h x", c=2, x=HW)[:, :, :, 0:64]
                ps_v = ps[:, :, :].rearrange("p c (h x) -> p c h x", x=64)
                bv_v = bvB_t[:, :].rearrange("p (c h x) -> p c h x", c=2, x=64)
                nc.vector.tensor_add(dst, ps_v, bv_v)

            def wblk_load(w_d, m):
                wb = wbp.tile([128, NB, 128], bf16, tag="wblk")
                nc.sync.dma_start(out=wb[:, :, :], in_=w_d.ap()[:, m, :, :])
                return wb

            def proj_block(wb, x_t, b_t, m):
                """[e-block m, s] = W.T-block @ x.T (+ bias) -> f32 tile.
                Kept in f32 so the scores matmuls can run in float32r
                (full-rate for moving dim >= 256) for better accuracy."""
                ps = vpp.tile([128, 2, 512], f32, tag="vp")
                for n in range(NB):
                    for c in range(2):
                        nc.tensor.matmul(
                            ps[:, c, :], wb[:, n, :], x_t[:, n, c * 512:(c + 1) * 512],
                            start=(n == 0), stop=(n == NB - 1),
                        )
                ob = qkp.tile([128, S], f32r, tag="qk")
                nc.vector.tensor_scalar_add(
                    ob[:, :].rearrange("p (c s) -> p c s", c=2), ps[:, :, :], b_t[:, m:m + 1])
                return ob

            def head(h, q1b, k1b, defer_norm=False, mid_cb=None):
                po = (h % 2) * 64
                pt = ptp.tile([128, NB, S], bf16, tag="pt")
                vps = vpp.tile([65, 2, 512], f32, tag="vp")

                def pv_chunk(n):
                    for c in range(2):
                        nc.tensor.matmul(
                            vps[:, c, :],
                            v1e[:, n, h * HW:(h + 1) * HW],
                            pt[:, n, c * 512:(c + 1) * 512],
                            start=(n == 0), stop=(n == NB - 1),
                        )

                # interleave PV fill in 2-chunk bursts between scores chunks
                # (fewer PE context switches than per-chunk interleave)
                for n in range(NB):
                    sps = mmp.tile([128, 2, 512], f32, tag="mm")
                    for c in range(2):
                        nc.tensor.matmul(
                            sps[:, c, :],
                            k1b[po:po + 64, n * 128:(n + 1) * 128],
                            q1b[po:po + 64, c * 512:(c + 1) * 512],
                            start=True, stop=True,
                        )
                    nc.scalar.activation(
                        out=pt[:, n, :].rearrange("p (c s) -> p c s", c=2),
                        in_=sps[:, :, :], func=ExpF, scale=0.125)
                    if n in (3, 5, 7):
                        pv_chunk(n - 3)
                        pv_chunk(n - 2)
                if mid_cb is not None:
                    mid_cb()
                for n in range(NB - 2, NB):
                    pv_chunk(n)

                if h < H - 1:
                    # DRAM-bounce partition broadcast of 1/denom (off critical
                    # path for all but the last heads)
                    rc = rcp.tile([65, S], f32, tag="rc")
                    nc.vector.reciprocal(
                        out=rc[64:65, :].rearrange("p (c s) -> p c s", c=2),
                        in_=vps[64:65, :, :])
                    nc.gpsimd.dma_start(out=rdram.ap()[h:h + 1, :], in_=rc[64:65, :])
                    rb = rcp.tile([64, S], f32, tag="rb")
                    nc.gpsimd.dma_start(
                        out=rb[:, :], in_=rdram.ap()[h:h + 1, :].to_broadcast((64, S)))
                else:
                    # last head pair feeds the output projection directly:
                    # use the shorter PE-broadcast chain (K=1 matmul) instead
                    rc = rcp.tile([65, S], f32, tag="rc")
                    nc.vector.reciprocal(
                        out=rc[64:65, :].rearrange("p (c s) -> p c s", c=2),
                        in_=vps[64:65, :, :])
                    rcb = rcp.tile([65, S], bf16, tag="rcb")
                    nc.vector.tensor_copy(rcb[64:65, :], rc[64:65, :])
                    rbp = mmp.tile([64, 2, 512], f32, tag="mm")
                    for c in range(2):
                        nc.tensor.matmul(
                            rbp[:, c, :], ones64[64:65, 0:64],
                            rcb[64:65, c * 512:(c + 1) * 512],
                            start=True, stop=True)
                    rb = rcp.tile([64, S], f32, tag="rb")
                    nc.vector.tensor_copy(
                        rb[:, :].rearrange("p (c s) -> p c s", c=2), rbp[:, :, :])

                m = h // 2

                def finish():
                    if h % 2 == 0:
                        dst = a1[m][0:64, :]
                        st = None
                    else:
                        st = stp.tile([64, S], bf16, tag="st")
                        dst = st[:, :]
                    nc.vector.tensor_mul(
                        dst.rearrange("p (c s) -> p c s", c=2),
                        vps[0:64, :, :],
                        rb[:, :].rearrange("p (c s) -> p c s", c=2))
                    if st is not None:
                        if h == H - 1:
                            nc.sync.dma_start(out=a1[m][64:128, :], in_=st[:, :])
                        else:
                            nc.gpsimd.dma_start(out=a1[m][64:128, :], in_=st[:, :])

                if defer_norm:
                    return finish
                finish()

            # ---- V projection (with Q0/K0 interleaved near the end so their
            # PSUM->SBUF drains hide under the remaining V-proj blocks) ----
            wqb = wblk_load(wq_d, 0)
            wkb = wblk_load(wk_d, 0)
            wc_t = wfp.tile([128, NB, NB, 128], bf16, tag="wc")
            nc.sync.dma_start(out=wc_t[:, :, :, :], in_=wc_d.ap())
            for n2 in range(NB - 2):
                vproj_block(n2)
            q1b = proj_block(wqb, xq_t, bq_t, 0)
            vproj_block(NB - 2)
            k1b = proj_block(wkb, xk_t, bk_t, 0)
            vproj_block(NB - 1)
            for m in range(NB):
                if m < NB - 1:
                    head(2 * m, q1b, k1b)
                    nwqb = wblk_load(wq_d, m + 1)
                    nwkb = wblk_load(wk_d, m + 1)
                    nq1b = proj_block(nwqb, xq_t, bq_t, m + 1)
                    nk1b = proj_block(nwkb, xk_t, bk_t, m + 1)
                    head(2 * m + 1, q1b, k1b)
                    q1b, k1b = nq1b, nk1b
                else:
                    # last pair: defer head-14's normalize multiply so its
                    # DRAM-bounce hides under head-15's scores, then finish it
                    # mid-head-15 (keeps the a1[7] tail chain short)
                    fin14 = head(2 * m, q1b, k1b, defer_norm=True)
                    head(2 * m + 1, q1b, k1b, mid_cb=fin14)

            # ---- output projection ----
            for m in range(NB):
                ops = mmp.tile([128, 2, 512], f32, tag="mm")
                for n in range(NB):
                    for c in range(2):
                        nc.tensor.matmul(
                            ops[:, c, :], wc_t[:, m, n, :], a1[n][:, c * 512:(c + 1) * 512],
                            start=(n == 0), stop=(n == NB - 1),
                        )
                if m < NB - 1:
                    ot = ostp.tile([128, 2, 512], f32, tag="ost")
                    nc.vector.tensor_scalar_add(ot[:, :, :], ops[:, :, :], bc_t[:, m:m + 1])
                    nc.sync.dma_start(
                        out=out_d.ap()[m * 128:(m + 1) * 128, :].rearrange(
                            "p (c s) -> p c s", c=2),
                        in_=ot[:, :, :])
                else:
                    # split the last store so its drain+DMA chain pipelines
                    for c in range(2):
                        ot = ostp.tile([128, 512], f32, tag="ostl")
                        nc.vector.tensor_scalar_add(ot[:, :], ops[:, c, :], bc_t[:, m:m + 1])
                        nc.sync.dma_start(
                            out=out_d.ap()[m * 128:(m + 1) * 128,
                                           c * 512:(c + 1) * 512],
                            in_=ot[:, :])

    nc.compile()
    return nc


def _get_nc():
    global _compiled
    if _compiled is None:
        _compiled = _build()
    return _compiled


def _make_in_maps(q, k, v, Wq, bq, Wk, bk, Wv, bv, Wq2, bq2, Wk2, bk2, Wv2, bv2,
                  Wc, bc, Wc2, bc2):
    bf16 = ml_dtypes.bfloat16

    def xpack(x):  # [s, d] -> [p, n, s]
        x = np.asarray(x, np.float32)
        return np.ascontiguousarray(x.reshape(S, NB, 128).transpose(2, 1, 0)).astype(bf16)

    def wpack(w):  # W[e, d] -> [p, m, n, e']
        w = np.asarray(w, np.float32)
        return np.ascontiguousarray(
            w.reshape(NB, 128, NB, 128).transpose(3, 0, 2, 1)).astype(bf16)

    def wvpack(w):  # Wv[e, d] -> [p, n, e]
        w = np.asarray(w, np.float32)
        return np.ascontiguousarray(w.T.reshape(NB, 128, D).transpose(1, 0, 2)).astype(bf16)

    def btile(b):
        return np.ascontiguousarray(np.asarray(b, np.float32).reshape(NB, 128).T)

    def brep(b):
        return np.ascontiguousarray(
            np.broadcast_to(np.asarray(b, np.float32), (128, D))).astype(bf16)

    paths = [
        dict(wq=wpack(Wq), wk=wpack(Wk), wv=wvpack(Wv), wc=wpack(Wc),
             bq=btile(bq), bk=btile(bk), bc=btile(bc), bvB=brep(bv)),
        dict(wq=wpack(Wq2), wk=wpack(Wk2), wv=wvpack(Wv2), wc=wpack(Wc2),
             bq=btile(bq2), bk=btile(bk2), bc=btile(bc2), bvB=brep(bv2)),
    ]
    in_maps = []
    for c in range(8):
        p, b = c // 4, c % 4
        if p == 0:
            xq, xk, xv = xpack(q[b]), xpack(k[b]), xpack(v[b])
        else:
            # path 2: q2 from k; k2, v2 from q
            xq, xk, xv = xpack(k[b]), xpack(q[b]), xpack(q[b])
        in_maps.append(dict(paths[p], xq=xq, xk=xk, xv=xv))
    return in_maps


def _run(in_maps, trace=False):
    from concourse.bass_utils import run_bass_kernel_spmd
    nc = _get_nc()
    return run_bass_kernel_spmd(nc, in_maps, core_ids=list(range(8)), trace=trace)


def kernel(**inputs):
    in_maps = _make_in_maps(**inputs)
    try:
        res = _run(in_maps)
    except Exception:
        # transient NRT_EXEC_UNIT_UNRECOVERABLE has been observed when a
        # prior process crashed mid-execution; one retry reloads the NEFF
        res = _run(in_maps)
    out1 = np.stack([res.results[b]["outT"].T for b in range(4)]).astype(np.float32)
    out2 = np.stack([res.results[4 + b]["outT"].T for b in range(4)]).astype(np.float32)
    return out1, out2



# revision 3
# speedup vs baseline: 1.0632x; 1.0632x over previous
"""Dual-path multi-head attention on 8 trn2 NeuronCores.

Sharding: core c = (path p=c//4, batch b=c%4). Each core runs the full
pipeline for one path and one batch element: 3 input projections, 16-head
attention (S=1024, dh=64), output projection. No collectives.

Path 2 cross-wiring (q2 from k; k2,v2 from q) is handled purely by host-side
input routing - every core runs the identical SPMD program.

Device layouts (per core, all pre-packed on host for contiguous DMA runs):
  xq/xk/xv : [p, n, s]   = x.T blocked:  x[s, n*128+p]
  wq/wc    : [p, m, n, e'] = W[m*128+e', n*128+p]  (W.T blocked by out-block m)
  wv       : [p, n, e]   = Wv[e, n*128+p]
  Q/K projections compute Q1T/K1T = [e, s]; V projection computes V1 = [s, e]
  (x stationary), stored per head with a ones column (v1e, width 65).
  Scores are computed transposed (probs_T[sk, sq]); softmax is max-free.

Swapped PV: probs_T blocks are the STATIONARY operand and v1e the moving
operand, so each PV matmul outputs [sq=128, 65] - 65 cycles/instruction
instead of 1024 - and the softmax denominator (ones column of v1e) lands as
PSUM column 64, a per-partition scalar: normalize is a single DVE
tensor_scalar per sq-block, no partition-broadcast needed. The normalized
attention output a1T [sq, e] is transposed back to a1 [e, s] for the output
projection with PE transpose instructions (128x128 tiles via identity).
"""

import numpy as np
import ml_dtypes

B, S, D, H, DH = 4, 1024, 1024, 16, 64
NB = D // 128  # 8 partition-blocks
HW = 65  # head slot width in v1e (64 data + 1 ones col)

_compiled = None


def _build():
    import concourse.bass as bass
    import concourse.mybir as mybir
    import concourse.tile as tile
    from concourse import bacc

    dt = mybir.dt
    f32, bf16, f32r = dt.float32, dt.bfloat16, dt.float32r

    nc = bacc.Bacc("TRN2", target_bir_lowering=False, debug=False)

    xq_d = nc.dram_tensor("xq", [128, NB, S], bf16, kind="ExternalInput")
    xk_d = nc.dram_tensor("xk", [128, NB, S], bf16, kind="ExternalInput")
    xv_d = nc.dram_tensor("xv", [128, NB, S], bf16, kind="ExternalInput")
    wq_d = nc.dram_tensor("wq", [128, NB, NB, 128], bf16, kind="ExternalInput")
    wk_d = nc.dram_tensor("wk", [128, NB, NB, 128], bf16, kind="ExternalInput")
    wv_d = nc.dram_tensor("wv", [128, NB, D], bf16, kind="ExternalInput")
    wc_d = nc.dram_tensor("wc", [128, NB, NB, 128], bf16, kind="ExternalInput")
    bq_d = nc.dram_tensor("bq", [128, NB], f32, kind="ExternalInput")
    bk_d = nc.dram_tensor("bk", [128, NB], f32, kind="ExternalInput")
    bc_d = nc.dram_tensor("bc", [128, NB], f32, kind="ExternalInput")
    bvB_d = nc.dram_tensor("bvB", [128, D], bf16, kind="ExternalInput")
    id_d = nc.dram_tensor("ident", [128, 128], bf16, kind="ExternalInput")
    out_d = nc.dram_tensor("outT", [D, S], f32, kind="ExternalOutput")

    ExpF = mybir.ActivationFunctionType.Exp
    MULT = mybir.AluOpType.mult

    with tile.TileContext(nc) as tc:
        with tc.tile_pool(name="x", bufs=3) as xp, \
             tc.tile_pool(name="wfull", bufs=1) as wfp, \
             tc.tile_pool(name="wblk", bufs=4) as wbp, \
             tc.tile_pool(name="cst", bufs=1) as cp, \
             tc.tile_pool(name="qk", bufs=4) as qkp, \
             tc.tile_pool(name="pers", bufs=1) as prp, \
             tc.tile_pool(name="pt", bufs=2) as ptp, \
             tc.tile_pool(name="a1t", bufs=2) as atp, \
             tc.tile_pool(name="rcs", bufs=4) as rcp, \
             tc.tile_pool(name="ost", bufs=2) as ostp, \
             tc.tile_pool(name="mm", bufs=2, space="PSUM") as mmp, \
             tc.tile_pool(name="vp", bufs=1, space="PSUM") as vpp, \
             tc.tile_pool(name="pv", bufs=2, space="PSUM") as pvp:

            # ---- loads: first V-proj blocks (interleaved for early start),
            # constants after the first block pair, then xq/xk; wc last.
            xv_t = xp.tile([128, NB, S], bf16, tag="x")
            wv_t = wfp.tile([128, NB, D], bf16)
            nc.sync.dma_start(out=xv_t[:, 0, :], in_=xv_d.ap()[:, 0, :])
            nc.sync.dma_start(out=wv_t[:, 0, :], in_=wv_d.ap()[:, 0, :])
            bq_t = cp.tile([128, NB], f32)
            nc.sync.dma_start(out=bq_t[:, :], in_=bq_d.ap())
            bk_t = cp.tile([128, NB], f32)
            nc.sync.dma_start(out=bk_t[:, :], in_=bk_d.ap())
            bc_t = cp.tile([128, NB], f32)
            nc.sync.dma_start(out=bc_t[:, :], in_=bc_d.ap())
            bvB_t = cp.tile([128, D], bf16)
            nc.sync.dma_start(out=bvB_t[:, :], in_=bvB_d.ap())
            id_t = cp.tile([128, 128], bf16)
            nc.sync.dma_start(out=id_t[:, :], in_=id_d.ap())
            for n in range(1, NB):
                nc.sync.dma_start(out=xv_t[:, n, :], in_=xv_d.ap()[:, n, :])
                nc.sync.dma_start(out=wv_t[:, n, :], in_=wv_d.ap()[:, n, :])
            xq_t = xp.tile([128, NB, S], bf16, tag="x")
            nc.sync.dma_start(out=xq_t[:, :, :], in_=xq_d.ap())
            xk_t = xp.tile([128, NB, S], bf16, tag="x")
            nc.sync.dma_start(out=xk_t[:, :, :], in_=xk_d.ap())

            v1e = prp.tile([128, NB, H * HW], bf16)
            a1 = [prp.tile([128, S], bf16, tag=f"a1_{n}", name=f"a1_{n}")
                  for n in range(NB)]

            # ones columns of v1e (softmax denominator trick)
            ones_ap = v1e[:, :, :].rearrange("p n (h x) -> p n h x", x=HW)[:, :, :, 64]
            nc.vector.memset(ones_ap, 1.0)

            def vproj_block(n2):
                ps = vpp.tile([128, 2, 512], f32, tag="vp", name=f"vps{n2}")
                for n in range(NB):
                    for c in range(2):
                        nc.tensor.matmul(
                            ps[:, c, :],
                            xv_t[:, n, n2 * 128:(n2 + 1) * 128],
                            wv_t[:, n, c * 512:(c + 1) * 512],
                            start=(n == 0), stop=(n == NB - 1),
                        )
                dst = v1e[:, n2, :].rearrange("p (c h x) -> p c h x", c=2, x=HW)[:, :, :, 0:64]
                ps_v = ps[:, :, :].rearrange("p c (h x) -> p c h x", x=64)
                bv_v = bvB_t[:, :].rearrange("p (c h x) -> p c h x", c=2, x=64)
                nc.vector.tensor_add(dst, ps_v, bv_v)

            def wblk_load(w_d, m):
                wb = wbp.tile([128, NB, 128], bf16, tag="wblk")
                nc.sync.dma_start(out=wb[:, :, :], in_=w_d.ap()[:, m, :, :])
                return wb

            def proj_block(wb, x_t, b_t, m):
                """[e-block m, s] = W.T-block @ x.T (+ bias) -> f32 tile.
                Kept in f32 so the scores matmuls can run in float32r
                (full-rate for moving dim >= 256) for better accuracy."""
                ps = vpp.tile([128, 2, 512], f32, tag="vp")
                for n in range(NB):
                    for c in range(2):
                        nc.tensor.matmul(
                            ps[:, c, :], wb[:, n, :], x_t[:, n, c * 512:(c + 1) * 512],
                            start=(n == 0), stop=(n == NB - 1),
                        )
                ob = qkp.tile([128, S], f32r, tag="qk")
                nc.vector.tensor_scalar_add(
                    ob[:, :].rearrange("p (c s) -> p c s", c=2), ps[:, :, :], b_t[:, m:m + 1])
                return ob

            def head(h, q1b, k1b, a1T, pre_cb=None):
                """Scores + exp + swapped PV + per-partition normalize.
                Writes normalized a1T[:, b, po:po+64] for all 8 sq-blocks b."""
                po = (h % 2) * 64
                pt = ptp.tile([128, NB, S], bf16, tag="pt")
                pva = pvp.tile([128, 4, HW], f32, tag="pv")
                pvb = pvp.tile([128, 4, HW], f32, tag="pv")
                # psum start=True zeroes the full 2KB bank, which would clobber
                # the other sq-block groups sharing it - memset + accumulate
                nc.vector.memset(pva[:, :, :], 0.0)
                nc.vector.memset(pvb[:, :, :], 0.0)

                def pv_round(n):
                    # swapped PV: probs_T block stationary, v1e moving
                    for b in range(NB):
                        dst = pva if b < 4 else pvb
                        nc.tensor.matmul(
                            dst[:, b % 4, :],
                            pt[:, n, b * 128:(b + 1) * 128],
                            v1e[:, n, h * HW:(h + 1) * HW],
                            start=False, stop=(n == NB - 1),
                            skip_group_check=True,
                        )

                for n in range(NB):
                    sps = mmp.tile([128, 2, 512], f32, tag="mm")
                    for c in range(2):
                        nc.tensor.matmul(
                            sps[:, c, :],
                            k1b[po:po + 64, n * 128:(n + 1) * 128],
                            q1b[po:po + 64, c * 512:(c + 1) * 512],
                            start=True, stop=True,
                        )
                    nc.scalar.activation(
                        out=pt[:, n, :].rearrange("p (c s) -> p c s", c=2),
                        in_=sps[:, :, :], func=ExpF, scale=0.125)
                    if n == 1 and pre_cb is not None:
                        # previous pair's transposes, hidden under this head
                        pre_cb()
                    if n in (3, 5, 7):
                        pv_round(n - 3)
                        pv_round(n - 2)
                pv_round(NB - 2)
                pv_round(NB - 1)

                # normalize: denominator is PSUM column 64 (per-partition)
                rc = rcp.tile([128, 2, 4], f32, tag="rc")
                nc.vector.reciprocal(out=rc[:, 0, :], in_=pva[:, :, 64])
                nc.vector.reciprocal(out=rc[:, 1, :], in_=pvb[:, :, 64])
                for b in range(NB):
                    src = pva if b < 4 else pvb
                    nc.vector.tensor_scalar(
                        a1T[:, b, po:po + 64], src[:, b % 4, 0:64],
                        rc[:, b // 4, b % 4:b % 4 + 1], None, MULT)

            def transposes(m, a1T):
                # a1T [sq, e-pair] -> a1[m] [e, s] via PE transpose + DVE copy
                tp = pvp.tile([128, 1024], bf16, tag="pv")
                for b in range(NB):
                    nc.tensor.transpose(
                        tp[:, b * 128:(b + 1) * 128], a1T[:, b, :], id_t[:, :])
                nc.vector.tensor_copy(a1[m][:, :], tp[:, :])

            # ---- V projection (with Q0/K0 interleaved near the end so their
            # PSUM->SBUF drains hide under the remaining V-proj blocks) ----
            wqb = wblk_load(wq_d, 0)
            wkb = wblk_load(wk_d, 0)
            wc_t = wfp.tile([128, NB, NB, 128], bf16, tag="wc")
            nc.sync.dma_start(out=wc_t[:, :, :, :], in_=wc_d.ap())
            for n2 in range(NB - 2):
                vproj_block(n2)
            q1b = proj_block(wqb, xq_t, bq_t, 0)
            vproj_block(NB - 2)
            k1b = proj_block(wkb, xk_t, bk_t, 0)
            vproj_block(NB - 1)

            pend = None
            for m in range(NB):
                a1T = atp.tile([128, NB, 128], bf16, tag="a1T")
                head(2 * m, q1b, k1b, a1T, pre_cb=pend)
                if m < NB - 1:
                    nwqb = wblk_load(wq_d, m + 1)
                    nwkb = wblk_load(wk_d, m + 1)
                    nq1b = proj_block(nwqb, xq_t, bq_t, m + 1)
                    nk1b = proj_block(nwkb, xk_t, bk_t, m + 1)
                head(2 * m + 1, q1b, k1b, a1T)
                if m < NB - 1:
                    q1b, k1b = nq1b, nk1b
                    pend = (lambda mm_, t_: lambda: transposes(mm_, t_))(m, a1T)
                else:
                    transposes(m, a1T)

            # ---- output projection ----
            for m in range(NB):
                ops = mmp.tile([128, 2, 512], f32, tag="mm")
                for n in range(NB):
                    for c in range(2):
                        nc.tensor.matmul(
                            ops[:, c, :], wc_t[:, m, n, :], a1[n][:, c * 512:(c + 1) * 512],
                            start=(n == 0), stop=(n == NB - 1),
                        )
                if m < NB - 1:
                    ot = ostp.tile([128, 2, 512], f32, tag="ost")
                    nc.vector.tensor_scalar_add(ot[:, :, :], ops[:, :, :], bc_t[:, m:m + 1])
                    nc.sync.dma_start(
                        out=out_d.ap()[m * 128:(m + 1) * 128, :].rearrange(
                            "p (c s) -> p c s", c=2),
                        in_=ot[:, :, :])
                else:
                    # split the last store so its drain+DMA chain pipelines
                    for c in range(2):
                        ot = ostp.tile([128, 512], f32, tag="ostl")
                        nc.vector.tensor_scalar_add(ot[:, :], ops[:, c, :], bc_t[:, m:m + 1])
                        nc.sync.dma_start(
                            out=out_d.ap()[m * 128:(m + 1) * 128,
                                           c * 512:(c + 1) * 512],
                            in_=ot[:, :])

    nc.compile()
    return nc


def _get_nc():
    global _compiled
    if _compiled is None:
        _compiled = _build()
    return _compiled


def _make_in_maps(q, k, v, Wq, bq, Wk, bk, Wv, bv, Wq2, bq2, Wk2, bk2, Wv2, bv2,
                  Wc, bc, Wc2, bc2):
    bf16 = ml_dtypes.bfloat16

    def xpack(x):  # [s, d] -> [p, n, s]
        x = np.asarray(x, np.float32)
        return np.ascontiguousarray(x.reshape(S, NB, 128).transpose(2, 1, 0)).astype(bf16)

    def wpack(w):  # W[e, d] -> [p, m, n, e']
        w = np.asarray(w, np.float32)
        return np.ascontiguousarray(
            w.reshape(NB, 128, NB, 128).transpose(3, 0, 2, 1)).astype(bf16)

    def wvpack(w):  # Wv[e, d] -> [p, n, e]
        w = np.asarray(w, np.float32)
        return np.ascontiguousarray(w.T.reshape(NB, 128, D).transpose(1, 0, 2)).astype(bf16)

    def btile(b):
        return np.ascontiguousarray(np.asarray(b, np.float32).reshape(NB, 128).T)

    def brep(b):
        return np.ascontiguousarray(
            np.broadcast_to(np.asarray(b, np.float32), (128, D))).astype(bf16)

    ident = np.ascontiguousarray(np.eye(128, dtype=np.float32)).astype(bf16)

    paths = [
        dict(wq=wpack(Wq), wk=wpack(Wk), wv=wvpack(Wv), wc=wpack(Wc),
             bq=btile(bq), bk=btile(bk), bc=btile(bc), bvB=brep(bv), ident=ident),
        dict(wq=wpack(Wq2), wk=wpack(Wk2), wv=wvpack(Wv2), wc=wpack(Wc2),
             bq=btile(bq2), bk=btile(bk2), bc=btile(bc2), bvB=brep(bv2), ident=ident),
    ]
    in_maps = []
    for c in range(8):
        p, b = c // 4, c % 4
        if p == 0:
            xq, xk, xv = xpack(q[b]), xpack(k[b]), xpack(v[b])
        else:
            # path 2: q2 from k; k2, v2 from q
            xq, xk, xv = xpack(k[b]), xpack(q[b]), xpack(q[b])
        in_maps.append(dict(paths[p], xq=xq, xk=xk, xv=xv))
    return in_maps


def _run(in_maps, trace=False):
    from concourse.bass_utils import run_bass_kernel_spmd
    nc = _get_nc()
    return run_bass_kernel_spmd(nc, in_maps, core_ids=list(range(8)), trace=trace)


def kernel(**inputs):
    in_maps = _make_in_maps(**inputs)
    try:
        res = _run(in_maps)
    except Exception:
        # transient NRT_EXEC_UNIT_UNRECOVERABLE has been observed when a
        # prior process crashed mid-execution; one retry reloads the NEFF
        res = _run(in_maps)
    out1 = np.stack([res.results[b]["outT"].T for b in range(4)]).astype(np.float32)
    out2 = np.stack([res.results[4 + b]["outT"].T for b in range(4)]).astype(np.float32)
    return out1, out2


# revision 8
# speedup vs baseline: 1.0795x; 1.0154x over previous
"""Dual-path multi-head attention on 8 trn2 NeuronCores.

Sharding: core c = (path p=c//4, batch b=c%4). Each core runs the full
pipeline for one path and one batch element: 3 input projections, 16-head
attention (S=1024, dh=64), output projection. No collectives.

Path 2 cross-wiring (q2 from k; k2,v2 from q) is handled purely by host-side
input routing - every core runs the identical SPMD program.

Device layouts (per core, all pre-packed on host for contiguous DMA runs):
  xq/xk/xv : [p, n, s]   = x.T blocked:  x[s, n*128+p]
  wq/wc    : [p, m, n, e'] = W[m*128+e', n*128+p]  (W.T blocked by out-block m)
  wv       : [p, n, e]   = Wv[e, n*128+p]
  Q/K projections compute Q1T/K1T = [e, s]; V projection computes V1 = [s, e]
  (x stationary), stored per head with a ones column (v1e, width 65).
  Scores are computed transposed (probs_T[sk, sq]); softmax is max-free.

Swapped PV: probs_T blocks are the STATIONARY operand and v1e the moving
operand, so each PV matmul outputs [sq=128, 65] - 65 cycles/instruction
instead of 1024 - and the softmax denominator (ones column of v1e) lands as
PSUM column 64, a per-partition scalar: normalize is a single DVE
tensor_scalar per sq-block, no partition-broadcast needed. The normalized
attention output a1T [sq, e] is transposed back to a1 [e, s] for the output
projection with PE transpose instructions (128x128 tiles via identity).
"""

import numpy as np
import ml_dtypes

B, S, D, H, DH = 4, 1024, 1024, 16, 64
NB = D // 128  # 8 partition-blocks
HW = 65  # head slot width in v1e (64 data + 1 ones col)

_compiled = None


def _build():
    import concourse.bass as bass
    import concourse.mybir as mybir
    import concourse.tile as tile
    from concourse import bacc

    dt = mybir.dt
    f32, bf16, f32r = dt.float32, dt.bfloat16, dt.float32r

    nc = bacc.Bacc("TRN2", target_bir_lowering=False, debug=False)

    xq_d = nc.dram_tensor("xq", [128, NB, S], bf16, kind="ExternalInput")
    xk_d = nc.dram_tensor("xk", [128, NB, S], bf16, kind="ExternalInput")
    xv_d = nc.dram_tensor("xv", [128, NB, S], bf16, kind="ExternalInput")
    wq_d = nc.dram_tensor("wq", [128, NB, NB, 128], bf16, kind="ExternalInput")
    wk_d = nc.dram_tensor("wk", [128, NB, NB, 128], bf16, kind="ExternalInput")
    wv_d = nc.dram_tensor("wv", [128, NB, D], bf16, kind="ExternalInput")
    wc_d = nc.dram_tensor("wc", [128, NB, NB, 128], bf16, kind="ExternalInput")
    bq_d = nc.dram_tensor("bq", [128, NB], f32, kind="ExternalInput")
    bk_d = nc.dram_tensor("bk", [128, NB], f32, kind="ExternalInput")
    bc_d = nc.dram_tensor("bc", [128, NB], f32, kind="ExternalInput")
    bvB_d = nc.dram_tensor("bvB", [128, D], bf16, kind="ExternalInput")
    id_d = nc.dram_tensor("ident", [128, 128], bf16, kind="ExternalInput")
    out_d = nc.dram_tensor("outT", [D, S], f32, kind="ExternalOutput")

    ExpF = mybir.ActivationFunctionType.Exp
    MULT = mybir.AluOpType.mult

    with tile.TileContext(nc) as tc:
        with tc.tile_pool(name="x", bufs=3) as xp, \
             tc.tile_pool(name="wfull", bufs=1) as wfp, \
             tc.tile_pool(name="wblk", bufs=4) as wbp, \
             tc.tile_pool(name="cst", bufs=1) as cp, \
             tc.tile_pool(name="qk", bufs=4) as qkp, \
             tc.tile_pool(name="pers", bufs=1) as prp, \
             tc.tile_pool(name="pt", bufs=2) as ptp, \
             tc.tile_pool(name="a1t", bufs=2) as atp, \
             tc.tile_pool(name="rcs", bufs=4) as rcp, \
             tc.tile_pool(name="ost", bufs=2) as ostp, \
             tc.tile_pool(name="mm", bufs=2, space="PSUM") as mmp, \
             tc.tile_pool(name="vp", bufs=2, space="PSUM") as vpp, \
             tc.tile_pool(name="pv", bufs=2, space="PSUM") as pvp:

            # ---- loads on 4 parallel DMA queues (sync/vector/gpsimd/scalar)
            # so Q0/K0 inputs, V-proj inputs and constants stream concurrently
            wqb0 = wbp.tile([128, NB, 128], bf16, tag="wblk")
            nc.sync.dma_start(out=wqb0[:, :, :], in_=wq_d.ap()[:, 0, :, :])
            wkb0 = wbp.tile([128, NB, 128], bf16, tag="wblk")
            nc.scalar.dma_start(out=wkb0[:, :, :], in_=wk_d.ap()[:, 0, :, :])
            xq_t = xp.tile([128, NB, S], bf16, tag="x")
            nc.sync.dma_start(out=xq_t[:, :, :], in_=xq_d.ap())
            xk_t = xp.tile([128, NB, S], bf16, tag="x")
            nc.scalar.dma_start(out=xk_t[:, :, :], in_=xk_d.ap())
            xv_t = xp.tile([128, NB, S], bf16, tag="x")
            nc.gpsimd.dma_start(out=xv_t[:, :, :], in_=xv_d.ap())
            bq_t = cp.tile([128, NB], f32)
            nc.gpsimd.dma_start(out=bq_t[:, :], in_=bq_d.ap())
            bk_t = cp.tile([128, NB], f32)
            nc.gpsimd.dma_start(out=bk_t[:, :], in_=bk_d.ap())
            bc_t = cp.tile([128, NB], f32)
            nc.gpsimd.dma_start(out=bc_t[:, :], in_=bc_d.ap())
            id_t = cp.tile([128, 128], bf16)
            nc.gpsimd.dma_start(out=id_t[:, :], in_=id_d.ap())
            wv_t = wfp.tile([128, NB, D], bf16)
            nc.sync.dma_start(out=wv_t[:, :, 0:512], in_=wv_d.ap()[:, :, 0:512])
            nc.scalar.dma_start(out=wv_t[:, :, 512:1024], in_=wv_d.ap()[:, :, 512:1024])
            bvB_t = cp.tile([128, D], bf16)
            nc.gpsimd.dma_start(out=bvB_t[:, :], in_=bvB_d.ap())

            v1e = prp.tile([128, NB, H * HW], bf16)
            a1 = [prp.tile([128, S], bf16, tag=f"a1_{n}", name=f"a1_{n}")
                  for n in range(NB)]

            # ones columns of v1e (softmax denominator trick)
            ones_ap = v1e[:, :, :].rearrange("p n (h x) -> p n h x", x=HW)[:, :, :, 64]
            nc.vector.memset(ones_ap, 1.0)

            def vproj_chunk(n2, c):
                """V-proj half-block: out [s-block n2, e-half c] + bias -> v1e."""
                ps = vpp.tile([128, 512], f32, tag="vp")
                for n in range(NB):
                    nc.tensor.matmul(
                        ps[:, :],
                        xv_t[:, n, n2 * 128:(n2 + 1) * 128],
                        wv_t[:, n, c * 512:(c + 1) * 512],
                        start=(n == 0), stop=(n == NB - 1),
                    )
                dst = v1e[:, n2, c * 8 * HW:(c + 1) * 8 * HW].rearrange(
                    "p (h x) -> p h x", x=HW)[:, :, 0:64]
                ps_v = ps[:, :].rearrange("p (h x) -> p h x", x=64)
                bv_v = bvB_t[:, c * 512:(c + 1) * 512].rearrange("p (h x) -> p h x", x=64)
                nc.vector.tensor_add(dst, ps_v, bv_v)

            def wblk_load(w_d, m, eng):
                wb = wbp.tile([128, NB, 128], bf16, tag="wblk")
                eng.dma_start(out=wb[:, :, :], in_=w_d.ap()[:, m, :, :])
                return wb

            def proj_chunk(wb, x_t, b_t, m, ob, c):
                """Half of [e-block m, s] = W.T-block @ x.T (+ bias).
                Kept in f32 so the scores matmuls can run in float32r
                (full-rate for moving dim >= 256) for better accuracy."""
                ps = vpp.tile([128, 512], f32, tag="vp")
                for n in range(NB):
                    nc.tensor.matmul(
                        ps[:, :], wb[:, n, :], x_t[:, n, c * 512:(c + 1) * 512],
                        start=(n == 0), stop=(n == NB - 1),
                    )
                nc.vector.tensor_scalar_add(
                    ob[:, c * 512:(c + 1) * 512], ps[:, :], b_t[:, m:m + 1])

            def head(h, q1b, k1b, a1T, fillers=None):
                """Scores + exp + swapped PV + per-partition normalize.
                Writes normalized a1T[:, b, po:po+64] for all 8 sq-blocks b.
                fillers: dict n -> [closure] of PE work to emit at slot n so
                the PE never idles while Act catches up on exp."""
                po = (h % 2) * 64
                pt = ptp.tile([128, NB, S], bf16, tag="pt")
                pva = pvp.tile([128, 4, HW], f32, tag="pv")
                pvb = pvp.tile([128, 4, HW], f32, tag="pv")
                # psum start=True zeroes the full 2KB bank, which would clobber
                # the other sq-block groups sharing it - memset + accumulate
                nc.vector.memset(pva[:, :, :], 0.0)
                nc.vector.memset(pvb[:, :, :], 0.0)

                def pv_round(n):
                    # swapped PV: probs_T block stationary, v1e moving
                    for b in range(NB):
                        dst = pva if b < 4 else pvb
                        nc.tensor.matmul(
                            dst[:, b % 4, :],
                            pt[:, n, b * 128:(b + 1) * 128],
                            v1e[:, n, h * HW:(h + 1) * HW],
                            start=False, stop=(n == NB - 1),
                            skip_group_check=True,
                        )

                for n in range(NB):
                    sps = mmp.tile([128, 2, 512], f32, tag="mm")
                    for c in range(2):
                        nc.tensor.matmul(
                            sps[:, c, :],
                            k1b[po:po + 64, n * 128:(n + 1) * 128],
                            q1b[po:po + 64, c * 512:(c + 1) * 512],
                            start=True, stop=True,
                        )
                    nc.scalar.activation(
                        out=pt[:, n, :].rearrange("p (c s) -> p c s", c=2),
                        in_=sps[:, :, :], func=ExpF, scale=0.125)
                    if fillers and n in fillers:
                        for f in fillers[n]:
                            f()
                    if n in (3, 5, 7):
                        pv_round(n - 3)
                        pv_round(n - 2)
                pv_round(NB - 2)
                pv_round(NB - 1)

                # normalize: denominator is PSUM column 64 (per-partition)
                rc = rcp.tile([128, 2, 4], f32, tag="rc")
                nc.vector.reciprocal(out=rc[:, 0, :], in_=pva[:, :, 64])
                nc.vector.reciprocal(out=rc[:, 1, :], in_=pvb[:, :, 64])
                for b in range(NB):
                    src = pva if b < 4 else pvb
                    nc.vector.tensor_scalar(
                        a1T[:, b, po:po + 64], src[:, b % 4, 0:64],
                        rc[:, b // 4, b % 4:b % 4 + 1], None, MULT)

            def transposes(m, a1T):
                # a1T [sq, e-pair] -> a1[m] [e, s] via PE transpose + DVE copy
                tp = pvp.tile([128, 1024], bf16, tag="pv")
                for b in range(NB):
                    nc.tensor.transpose(
                        tp[:, b * 128:(b + 1) * 128], a1T[:, b, :], id_t[:, :])
                nc.vector.tensor_copy(a1[m][:, :], tp[:, :])

            # ---- Q0/K0 first (heads start ASAP), V-proj + later Q/K blocks
            # woven into the head loop as PE filler work ----
            q1b = qkp.tile([128, S], f32r, tag="qk")
            proj_chunk(wqb0, xq_t, bq_t, 0, q1b, 0)
            proj_chunk(wqb0, xq_t, bq_t, 0, q1b, 1)
            k1b = qkp.tile([128, S], f32r, tag="qk")
            proj_chunk(wkb0, xk_t, bk_t, 0, k1b, 0)
            proj_chunk(wkb0, xk_t, bk_t, 0, k1b, 1)
            vproj_chunk(0, 0)
            vproj_chunk(0, 1)
            vproj_chunk(1, 0)
            vproj_chunk(1, 1)

            def vpf(n2, c):
                return lambda: vproj_chunk(n2, c)

            wc_t = wfp.tile([128, NB, NB, 128], bf16, tag="wc")
            state = {}

            def load_next(m):
                def f():
                    state['wqb'] = wblk_load(wq_d, m + 1, nc.sync)
                    state['wkb'] = wblk_load(wk_d, m + 1, nc.scalar)
                    state['q1b'] = qkp.tile([128, S], f32r, tag="qk", name=f"q1b{m+1}")
                    state['k1b'] = qkp.tile([128, S], f32r, tag="qk", name=f"k1b{m+1}")
                return f

            def pcq(m, c):
                return lambda: proj_chunk(state['wqb'], xq_t, bq_t, m, state['q1b'], c)

            def pck(m, c):
                return lambda: proj_chunk(state['wkb'], xk_t, bk_t, m, state['k1b'], c)

            def tpose(m, t):
                return lambda: transposes(m, t)

            prev = None  # (m, a1T) of previous finished pair
            for m in range(NB):
                a1T = atp.tile([128, NB, 128], bf16, tag="a1T")
                if m == 0:
                    fe = {0: [vpf(2, 0), vpf(2, 1)], 1: [vpf(3, 0), vpf(3, 1)],
                          4: [vpf(4, 0), vpf(4, 1)], 6: [vpf(5, 0), vpf(5, 1), vpf(6, 0)],
                          7: [vpf(6, 1), vpf(7, 0), vpf(7, 1)]}
                    fo = {0: [load_next(0), pcq(1, 0)], 2: [pcq(1, 1)],
                          4: [pck(1, 0)], 6: [pck(1, 1)]}
                elif m < NB - 1:
                    fe = {0: [load_next(m)], 1: [tpose(*prev)], 2: [pcq(m + 1, 0)],
                          6: [pcq(m + 1, 1)]}
                    fo = {2: [pck(m + 1, 0)], 6: [pck(m + 1, 1)]}
                else:
                    fe = {1: [tpose(*prev)]}
                    fo = {}
                if m == 4:
                    # wc needed only by the output projection; queues idle now
                    fe.setdefault(0, []).append(lambda: (
                        nc.sync.dma_start(out=wc_t[:, 0:4, :, :], in_=wc_d.ap()[:, 0:4, :, :]),
                        nc.scalar.dma_start(out=wc_t[:, 4:8, :, :], in_=wc_d.ap()[:, 4:8, :, :])))
                head(2 * m, q1b, k1b, a1T, fillers=fe)
                head(2 * m + 1, q1b, k1b, a1T, fillers=fo)
                prev = (m, a1T)
                if m < NB - 1:
                    q1b, k1b = state['q1b'], state['k1b']
                else:
                    transposes(m, a1T)

            # ---- output projection ----
            for m in range(NB):
                ops = mmp.tile([128, 2, 512], f32, tag="mm")
                for n in range(NB):
                    for c in range(2):
                        nc.tensor.matmul(
                            ops[:, c, :], wc_t[:, m, n, :], a1[n][:, c * 512:(c + 1) * 512],
                            start=(n == 0), stop=(n == NB - 1),
                        )
                if m < NB - 1:
                    ot = ostp.tile([128, 2, 512], f32, tag="ost")
                    nc.vector.tensor_scalar_add(ot[:, :, :], ops[:, :, :], bc_t[:, m:m + 1])
                    nc.sync.dma_start(
                        out=out_d.ap()[m * 128:(m + 1) * 128, :].rearrange(
                            "p (c s) -> p c s", c=2),
                        in_=ot[:, :, :])
                else:
                    # split the last store so its drain+DMA chain pipelines
                    for c in range(2):
                        ot = ostp.tile([128, 512], f32, tag="ostl")
                        nc.vector.tensor_scalar_add(ot[:, :], ops[:, c, :], bc_t[:, m:m + 1])
                        nc.sync.dma_start(
                            out=out_d.ap()[m * 128:(m + 1) * 128,
                                           c * 512:(c + 1) * 512],
                            in_=ot[:, :])

    nc.compile()
    return nc


def _get_nc():
    global _compiled
    if _compiled is None:
        _compiled = _build()
    return _compiled


def _make_in_maps(q, k, v, Wq, bq, Wk, bk, Wv, bv, Wq2, bq2, Wk2, bk2, Wv2, bv2,
                  Wc, bc, Wc2, bc2):
    bf16 = ml_dtypes.bfloat16

    def xpack(x):  # [s, d] -> [p, n, s]
        x = np.asarray(x, np.float32)
        return np.ascontiguousarray(x.reshape(S, NB, 128).transpose(2, 1, 0)).astype(bf16)

    def wpack(w):  # W[e, d] -> [p, m, n, e']
        w = np.asarray(w, np.float32)
        return np.ascontiguousarray(
            w.reshape(NB, 128, NB, 128).transpose(3, 0, 2, 1)).astype(bf16)

    def wvpack(w):  # Wv[e, d] -> [p, n, e]
        w = np.asarray(w, np.float32)
        return np.ascontiguousarray(w.T.reshape(NB, 128, D).transpose(1, 0, 2)).astype(bf16)

    def btile(b):
        return np.ascontiguousarray(np.asarray(b, np.float32).reshape(NB, 128).T)

    def brep(b):
        return np.ascontiguousarray(
            np.broadcast_to(np.asarray(b, np.float32), (128, D))).astype(bf16)

    ident = np.ascontiguousarray(np.eye(128, dtype=np.float32)).astype(bf16)

    paths = [
        dict(wq=wpack(Wq), wk=wpack(Wk), wv=wvpack(Wv), wc=wpack(Wc),
             bq=btile(bq), bk=btile(bk), bc=btile(bc), bvB=brep(bv), ident=ident),
        dict(wq=wpack(Wq2), wk=wpack(Wk2), wv=wvpack(Wv2), wc=wpack(Wc2),
             bq=btile(bq2), bk=btile(bk2), bc=btile(bc2), bvB=brep(bv2), ident=ident),
    ]
    in_maps = []
    for c in range(8):
        p, b = c // 4, c % 4
        if p == 0:
            xq, xk, xv = xpack(q[b]), xpack(k[b]), xpack(v[b])
        else:
            # path 2: q2 from k; k2, v2 from q
            xq, xk, xv = xpack(k[b]), xpack(q[b]), xpack(q[b])
        in_maps.append(dict(paths[p], xq=xq, xk=xk, xv=xv))
    return in_maps


def _run(in_maps, trace=False):
    from concourse.bass_utils import run_bass_kernel_spmd
    nc = _get_nc()
    return run_bass_kernel_spmd(nc, in_maps, core_ids=list(range(8)), trace=trace)


def kernel(**inputs):
    in_maps = _make_in_maps(**inputs)
    try:
        res = _run(in_maps)
    except Exception:
        # transient NRT_EXEC_UNIT_UNRECOVERABLE has been observed when a
        # prior process crashed mid-execution; one retry reloads the NEFF
        res = _run(in_maps)
    out1 = np.stack([res.results[b]["outT"].T for b in range(4)]).astype(np.float32)
    out2 = np.stack([res.results[4 + b]["outT"].T for b in range(4)]).astype(np.float32)
    return out1, out2


# revision 10
# speedup vs baseline: 1.0880x; 1.0078x over previous
"""Dual-path multi-head attention on 8 trn2 NeuronCores.

Sharding: core c = (path p=c//4, batch b=c%4). Each core runs the full
pipeline for one path and one batch element: 3 input projections, 16-head
attention (S=1024, dh=64), output projection. No collectives.

Path 2 cross-wiring (q2 from k; k2,v2 from q) is handled purely by host-side
input routing - every core runs the identical SPMD program.

Device layouts (per core, all pre-packed on host for contiguous DMA runs):
  xq/xk/xv : [p, n, s]   = x.T blocked:  x[s, n*128+p]
  wq/wc    : [p, m, n, e'] = W[m*128+e', n*128+p]  (W.T blocked by out-block m)
  wv       : [p, n, e]   = Wv[e, n*128+p]
  Q/K projections compute Q1T/K1T = [e, s]; V projection computes V1 = [s, e]
  (x stationary), stored per head with a ones column (v1e, width 65).
  Scores are computed transposed (probs_T[sk, sq]); softmax is max-free.

Swapped PV: probs_T blocks are the STATIONARY operand and v1e the moving
operand, so each PV matmul outputs [sq=128, 65] - 65 cycles/instruction
instead of 1024 - and the softmax denominator (ones column of v1e) lands as
PSUM column 64, a per-partition scalar: normalize is a single DVE
tensor_scalar per sq-block, no partition-broadcast needed. The normalized
attention output a1T [sq, e] is transposed back to a1 [e, s] for the output
projection with PE transpose instructions (128x128 tiles via identity).
"""

import numpy as np
import ml_dtypes

B, S, D, H, DH = 4, 1024, 1024, 16, 64
NB = D // 128  # 8 partition-blocks
HW = 65  # head slot width in v1e (64 data + 1 ones col)

_compiled = None


def _build():
    import concourse.bass as bass
    import concourse.mybir as mybir
    import concourse.tile as tile
    from concourse import bacc

    dt = mybir.dt
    f32, bf16, f32r = dt.float32, dt.bfloat16, dt.float32r

    nc = bacc.Bacc("TRN2", target_bir_lowering=False, debug=False)

    xq_d = nc.dram_tensor("xq", [128, NB, S], bf16, kind="ExternalInput")
    xk_d = nc.dram_tensor("xk", [128, NB, S], bf16, kind="ExternalInput")
    xv_d = nc.dram_tensor("xv", [128, NB, S], bf16, kind="ExternalInput")
    wq_d = nc.dram_tensor("wq", [128, NB, NB, 128], bf16, kind="ExternalInput")
    wk_d = nc.dram_tensor("wk", [128, NB, NB, 128], bf16, kind="ExternalInput")
    wv_d = nc.dram_tensor("wv", [128, NB, D], bf16, kind="ExternalInput")
    wc_d = nc.dram_tensor("wc", [128, NB, NB, 128], bf16, kind="ExternalInput")
    bq_d = nc.dram_tensor("bq", [128, NB], f32, kind="ExternalInput")
    bk_d = nc.dram_tensor("bk", [128, NB], f32, kind="ExternalInput")
    bc_d = nc.dram_tensor("bc", [128, NB], f32, kind="ExternalInput")
    bvB_d = nc.dram_tensor("bvB", [128, D], bf16, kind="ExternalInput")
    id_d = nc.dram_tensor("ident", [128, 128], bf16, kind="ExternalInput")
    out_d = nc.dram_tensor("outT", [D, S], f32, kind="ExternalOutput")

    ExpF = mybir.ActivationFunctionType.Exp
    MULT = mybir.AluOpType.mult

    with tile.TileContext(nc) as tc:
        with tc.tile_pool(name="x", bufs=3) as xp, \
             tc.tile_pool(name="wfull", bufs=1) as wfp, \
             tc.tile_pool(name="wblk", bufs=4) as wbp, \
             tc.tile_pool(name="cst", bufs=1) as cp, \
             tc.tile_pool(name="qk", bufs=4) as qkp, \
             tc.tile_pool(name="pers", bufs=1) as prp, \
             tc.tile_pool(name="pt", bufs=2) as ptp, \
             tc.tile_pool(name="a1t", bufs=2) as atp, \
             tc.tile_pool(name="rcs", bufs=4) as rcp, \
             tc.tile_pool(name="ost", bufs=2) as ostp, \
             tc.tile_pool(name="mm", bufs=2, space="PSUM") as mmp, \
             tc.tile_pool(name="vp", bufs=1, space="PSUM") as vpp, \
             tc.tile_pool(name="pv", bufs=3, space="PSUM") as pvp:

            # ---- loads on 4 parallel DMA queues (sync/vector/gpsimd/scalar)
            # so Q0/K0 inputs, V-proj inputs and constants stream concurrently
            wqb0 = wbp.tile([128, NB, 128], bf16, tag="wblk")
            nc.sync.dma_start(out=wqb0[:, :, :], in_=wq_d.ap()[:, 0, :, :])
            wkb0 = wbp.tile([128, NB, 128], bf16, tag="wblk")
            nc.scalar.dma_start(out=wkb0[:, :, :], in_=wk_d.ap()[:, 0, :, :])
            xq_t = xp.tile([128, NB, S], bf16, tag="x")
            nc.sync.dma_start(out=xq_t[:, 0:4, :], in_=xq_d.ap()[:, 0:4, :])
            nc.sync.dma_start(out=xq_t[:, 4:8, :], in_=xq_d.ap()[:, 4:8, :])
            xk_t = xp.tile([128, NB, S], bf16, tag="x")
            nc.scalar.dma_start(out=xk_t[:, 0:4, :], in_=xk_d.ap()[:, 0:4, :])
            nc.scalar.dma_start(out=xk_t[:, 4:8, :], in_=xk_d.ap()[:, 4:8, :])
            bq_t = cp.tile([128, NB], f32)
            nc.gpsimd.dma_start(out=bq_t[:, :], in_=bq_d.ap())
            bk_t = cp.tile([128, NB], f32)
            nc.gpsimd.dma_start(out=bk_t[:, :], in_=bk_d.ap())
            xv_t = xp.tile([128, NB, S], bf16, tag="x")
            nc.gpsimd.dma_start(out=xv_t[:, :, :], in_=xv_d.ap())
            bc_t = cp.tile([128, NB], f32)
            nc.gpsimd.dma_start(out=bc_t[:, :], in_=bc_d.ap())
            id_t = cp.tile([128, 128], bf16)
            nc.gpsimd.dma_start(out=id_t[:, :], in_=id_d.ap())
            wv_t = wfp.tile([128, NB, D], bf16)
            nc.sync.dma_start(out=wv_t[:, :, 0:512], in_=wv_d.ap()[:, :, 0:512])
            nc.scalar.dma_start(out=wv_t[:, :, 512:1024], in_=wv_d.ap()[:, :, 512:1024])
            bvB_t = cp.tile([128, D], bf16)
            nc.gpsimd.dma_start(out=bvB_t[:, :], in_=bvB_d.ap())

            v1e = prp.tile([128, NB, H * HW], bf16)
            a1 = [prp.tile([128, S], bf16, tag=f"a1_{n}", name=f"a1_{n}")
                  for n in range(NB)]

            # ones columns of v1e (softmax denominator trick)
            ones_ap = v1e[:, :, :].rearrange("p n (h x) -> p n h x", x=HW)[:, :, :, 64]
            nc.vector.memset(ones_ap, 1.0)

            def vproj_chunk(n2, c):
                """V-proj half-block: out [s-block n2, e-half c] + bias -> v1e."""
                ps = vpp.tile([128, 512], f32, tag="vp")
                for n in range(NB):
                    nc.tensor.matmul(
                        ps[:, :],
                        xv_t[:, n, n2 * 128:(n2 + 1) * 128],
                        wv_t[:, n, c * 512:(c + 1) * 512],
                        start=(n == 0), stop=(n == NB - 1),
                    )
                dst = v1e[:, n2, c * 8 * HW:(c + 1) * 8 * HW].rearrange(
                    "p (h x) -> p h x", x=HW)[:, :, 0:64]
                ps_v = ps[:, :].rearrange("p (h x) -> p h x", x=64)
                bv_v = bvB_t[:, c * 512:(c + 1) * 512].rearrange("p (h x) -> p h x", x=64)
                nc.vector.tensor_add(dst, ps_v, bv_v)

            def wblk_load(w_d, m, eng):
                wb = wbp.tile([128, NB, 128], bf16, tag="wblk")
                eng.dma_start(out=wb[:, :, :], in_=w_d.ap()[:, m, :, :])
                return wb

            def proj_chunk(wb, x_t, b_t, m, ob, c):
                """Half of [e-block m, s] = W.T-block @ x.T (+ bias).
                Kept in f32 so the scores matmuls can run in float32r
                (full-rate for moving dim >= 256) for better accuracy."""
                ps = vpp.tile([128, 512], f32, tag="vp")
                for n in range(NB):
                    nc.tensor.matmul(
                        ps[:, :], wb[:, n, :], x_t[:, n, c * 512:(c + 1) * 512],
                        start=(n == 0), stop=(n == NB - 1),
                    )
                nc.vector.tensor_scalar_add(
                    ob[:, c * 512:(c + 1) * 512], ps[:, :], b_t[:, m:m + 1])

            def head(h, q1b, k1b, prev, fillers=None):
                """Scores + exp for head h; the PREVIOUS head's PV rounds are
                interleaved (lag-one-head) so PV never waits on this head's
                exp. fillers: dict n -> [closures] of extra PE work."""
                po = (h % 2) * 64
                pt = ptp.tile([128, NB, S], bf16, tag="pt")
                for n in range(NB):
                    sps = mmp.tile([128, 2, 512], f32, tag="mm")
                    for c in range(2):
                        nc.tensor.matmul(
                            sps[:, c, :],
                            k1b[po:po + 64, n * 128:(n + 1) * 128],
                            q1b[po:po + 64, c * 512:(c + 1) * 512],
                            start=True, stop=True,
                        )
                    nc.scalar.activation(
                        out=pt[:, n, :].rearrange("p (c s) -> p c s", c=2),
                        in_=sps[:, :, :], func=ExpF, scale=0.125)
                    if fillers and n in fillers:
                        for f in fillers[n]:
                            f()
                    if prev is not None and n in (3, 5, 7):
                        prev['round'](n - 3)
                        prev['round'](n - 2)
                if prev is not None:
                    prev['round'](NB - 2)
                    prev['round'](NB - 1)
                    prev['fin']()
                return pt

            def make_pv(h, pt, a1T):
                """PV + normalize closures for head h (emitted in head h+1)."""
                po = (h % 2) * 64
                pva = pvp.tile([128, 4, HW], f32, tag="pv")
                pvb = pvp.tile([128, 4, HW], f32, tag="pv")
                # psum start=True zeroes the full 2KB bank, which would clobber
                # the other sq-block groups sharing it - memset + accumulate
                nc.vector.memset(pva[:, :, :], 0.0)
                nc.vector.memset(pvb[:, :, :], 0.0)

                def rnd(n):
                    # swapped PV: probs_T block stationary, v1e moving
                    for b in range(NB):
                        dst = pva if b < 4 else pvb
                        nc.tensor.matmul(
                            dst[:, b % 4, :],
                            pt[:, n, b * 128:(b + 1) * 128],
                            v1e[:, n, h * HW:(h + 1) * HW],
                            start=False, stop=(n == NB - 1),
                            skip_group_check=True,
                        )

                def fin():
                    # normalize: denominator is PSUM column 64 (per-partition)
                    rc = rcp.tile([128, 2, 4], f32, tag="rc")
                    nc.vector.reciprocal(out=rc[:, 0, :], in_=pva[:, :, 64])
                    nc.vector.reciprocal(out=rc[:, 1, :], in_=pvb[:, :, 64])
                    for b in range(NB):
                        src = pva if b < 4 else pvb
                        nc.vector.tensor_scalar(
                            a1T[:, b, po:po + 64], src[:, b % 4, 0:64],
                            rc[:, b // 4, b % 4:b % 4 + 1], None, MULT)

                return {'round': rnd, 'fin': fin}

            def transposes(m, a1T):
                # a1T [sq, e-pair] -> a1[m] [e, s] via PE transpose + DVE copy
                tp = pvp.tile([128, 1024], bf16, tag="pv")
                for b in range(NB):
                    nc.tensor.transpose(
                        tp[:, b * 128:(b + 1) * 128], a1T[:, b, :], id_t[:, :])
                nc.vector.tensor_copy(a1[m][:, :], tp[:, :])

            # ---- Q0/K0 first (heads start ASAP); V-proj and later Q/K blocks
            # woven into the head loop as PE filler work ----
            q1b = qkp.tile([128, S], f32r, tag="qk")
            proj_chunk(wqb0, xq_t, bq_t, 0, q1b, 0)
            proj_chunk(wqb0, xq_t, bq_t, 0, q1b, 1)
            k1b = qkp.tile([128, S], f32r, tag="qk")
            proj_chunk(wkb0, xk_t, bk_t, 0, k1b, 0)
            proj_chunk(wkb0, xk_t, bk_t, 0, k1b, 1)

            def vpf(n2, c):
                return lambda: vproj_chunk(n2, c)

            wc_t = wfp.tile([128, NB, NB, 128], bf16, tag="wc")
            state = {}

            def load_next(m):
                def f():
                    state['wqb'] = wblk_load(wq_d, m + 1, nc.sync)
                    state['wkb'] = wblk_load(wk_d, m + 1, nc.scalar)
                    state['q1b'] = qkp.tile([128, S], f32r, tag="qk", name=f"q1b{m+1}")
                    state['k1b'] = qkp.tile([128, S], f32r, tag="qk", name=f"k1b{m+1}")
                return f

            def pcq(m, c):
                return lambda: proj_chunk(state['wqb'], xq_t, bq_t, m, state['q1b'], c)

            def pck(m, c):
                return lambda: proj_chunk(state['wkb'], xk_t, bk_t, m, state['k1b'], c)

            def tpose(m, t):
                return lambda: transposes(m, t)

            def wc_load():
                def f():
                    nc.sync.dma_start(out=wc_t[:, 0:4, :, :], in_=wc_d.ap()[:, 0:4, :, :])
                    nc.scalar.dma_start(out=wc_t[:, 4:8, :, :], in_=wc_d.ap()[:, 4:8, :, :])
                return f

            # filler schedule: vproj in heads 0-1, Q/K(m+1) in heads 2m/2m+1,
            # transposes(m) at head 2m+3 slot 1 (after normalize of 2m+1 which
            # runs at the end of head 2m+2)
            fill = {h: {} for h in range(H)}
            fill[0] = {2: [vpf(0, 0), vpf(0, 1)], 3: [vpf(1, 0), vpf(1, 1)],
                       4: [vpf(2, 0)], 5: [vpf(2, 1)], 6: [vpf(3, 0)], 7: [vpf(3, 1)]}
            fill[1] = {0: [vpf(4, 0), vpf(4, 1)], 1: [vpf(5, 0), vpf(5, 1)],
                       2: [load_next(0), pcq(1, 0)], 3: [pcq(1, 1)],
                       4: [vpf(6, 0), vpf(6, 1)], 5: [pck(1, 0)],
                       6: [vpf(7, 0), vpf(7, 1)], 7: [pck(1, 1)]}
            for m in range(1, NB - 1):
                fill[2 * m][0] = [load_next(m)]
                fill[2 * m][2] = [pcq(m + 1, 0)]
                fill[2 * m][6] = [pcq(m + 1, 1)]
                fill[2 * m + 1][2] = [pck(m + 1, 0)]
                fill[2 * m + 1][6] = [pck(m + 1, 1)]
            fill[8].setdefault(0, []).append(wc_load())

            prev = None
            a1T = None
            for h in range(H):
                m = h // 2
                if h % 2 == 0:
                    a1T = atp.tile([128, NB, 128], bf16, tag="a1T", name=f"a1T{m}")
                if m >= 1 and h % 2 == 1:
                    # transpose pair m-1 (its normalize ran at end of head 2m)
                    fill[h].setdefault(1, []).append(tpose(m - 1, a1T_prev))
                cur_a1T = a1T
                pt = head(h, q1b, k1b, prev, fillers=fill[h])
                prev = make_pv(h, pt, cur_a1T)
                if h % 2 == 1:
                    a1T_prev = a1T
                    if m < NB - 1:
                        q1b, k1b = state['q1b'], state['k1b']

            # tail: head 15's PV + normalize + last pair transpose
            for r in range(NB):
                prev['round'](r)
            prev['fin']()
            transposes(NB - 1, a1T)

            # ---- output projection ----
            for m in range(NB):
                ops = mmp.tile([128, 2, 512], f32, tag="mm")
                for n in range(NB):
                    for c in range(2):
                        nc.tensor.matmul(
                            ops[:, c, :], wc_t[:, m, n, :], a1[n][:, c * 512:(c + 1) * 512],
                            start=(n == 0), stop=(n == NB - 1),
                        )
                if m < NB - 1:
                    ot = ostp.tile([128, 2, 512], f32, tag="ost")
                    nc.vector.tensor_scalar_add(ot[:, :, :], ops[:, :, :], bc_t[:, m:m + 1])
                    nc.sync.dma_start(
                        out=out_d.ap()[m * 128:(m + 1) * 128, :].rearrange(
                            "p (c s) -> p c s", c=2),
                        in_=ot[:, :, :])
                else:
                    # split the last store so its drain+DMA chain pipelines
                    for c in range(2):
                        ot = ostp.tile([128, 512], f32, tag="ostl")
                        nc.vector.tensor_scalar_add(ot[:, :], ops[:, c, :], bc_t[:, m:m + 1])
                        nc.sync.dma_start(
                            out=out_d.ap()[m * 128:(m + 1) * 128,
                                           c * 512:(c + 1) * 512],
                            in_=ot[:, :])

    nc.compile()
    return nc


def _get_nc():
    global _compiled
    if _compiled is None:
        _compiled = _build()
    return _compiled


def _make_in_maps(q, k, v, Wq, bq, Wk, bk, Wv, bv, Wq2, bq2, Wk2, bk2, Wv2, bv2,
                  Wc, bc, Wc2, bc2):
    bf16 = ml_dtypes.bfloat16

    def xpack(x):  # [s, d] -> [p, n, s]
        x = np.asarray(x, np.float32)
        return np.ascontiguousarray(x.reshape(S, NB, 128).transpose(2, 1, 0)).astype(bf16)

    def wpack(w):  # W[e, d] -> [p, m, n, e']
        w = np.asarray(w, np.float32)
        return np.ascontiguousarray(
            w.reshape(NB, 128, NB, 128).transpose(3, 0, 2, 1)).astype(bf16)

    def wvpack(w):  # Wv[e, d] -> [p, n, e]
        w = np.asarray(w, np.float32)
        return np.ascontiguousarray(w.T.reshape(NB, 128, D).transpose(1, 0, 2)).astype(bf16)

    def btile(b):
        return np.ascontiguousarray(np.asarray(b, np.float32).reshape(NB, 128).T)

    def brep(b):
        return np.ascontiguousarray(
            np.broadcast_to(np.asarray(b, np.float32), (128, D))).astype(bf16)

    ident = np.ascontiguousarray(np.eye(128, dtype=np.float32)).astype(bf16)

    paths = [
        dict(wq=wpack(Wq), wk=wpack(Wk), wv=wvpack(Wv), wc=wpack(Wc),
             bq=btile(bq), bk=btile(bk), bc=btile(bc), bvB=brep(bv), ident=ident),
        dict(wq=wpack(Wq2), wk=wpack(Wk2), wv=wvpack(Wv2), wc=wpack(Wc2),
             bq=btile(bq2), bk=btile(bk2), bc=btile(bc2), bvB=brep(bv2), ident=ident),
    ]
    in_maps = []
    for c in range(8):
        p, b = c // 4, c % 4
        if p == 0:
            xq, xk, xv = xpack(q[b]), xpack(k[b]), xpack(v[b])
        else:
            # path 2: q2 from k; k2, v2 from q
            xq, xk, xv = xpack(k[b]), xpack(q[b]), xpack(q[b])
        in_maps.append(dict(paths[p], xq=xq, xk=xk, xv=xv))
    return in_maps


def _run(in_maps, trace=False):
    from concourse.bass_utils import run_bass_kernel_spmd
    nc = _get_nc()
    return run_bass_kernel_spmd(nc, in_maps, core_ids=list(range(8)), trace=trace)


def kernel(**inputs):
    in_maps = _make_in_maps(**inputs)
    try:
        res = _run(in_maps)
    except Exception:
        # transient NRT_EXEC_UNIT_UNRECOVERABLE has been observed when a
        # prior process crashed mid-execution; one retry reloads the NEFF
        res = _run(in_maps)
    out1 = np.stack([res.results[b]["outT"].T for b in range(4)]).astype(np.float32)
    out2 = np.stack([res.results[4 + b]["outT"].T for b in range(4)]).astype(np.float32)
    return out1, out2
